# revision 1
# baseline (speedup 1.0000x reference)
"""Deformable Conv2d (modulated, torchvision v2 layout) on 8 Trainium2 NeuronCores.

Strategy: data-parallel over batch (B=8 -> 1 image per core).
Per core, entirely on device:
  1. offset/mask convs on TensorE (im2col-by-shifted-AP matmuls, fp32)
  2. transpose conv output to position-major [p, 27]; compute bilinear
     corner weights (x mask, x validity) and int32 quad-table indices on
     VectorE with batched strided APs
  3. 162 indirect DMAs gather 2KB bf16 "quad" rows (all 4 bilinear corners
     of one tap/position, pre-packed host-side with 2-pixel zero padding so
     out-of-bounds sampling is exactly zero like the reference)
  4. scalar_tensor_tensor combines the 4 corners with per-partition weights
  5. TensorE transposes vals to channel-major; contraction matmul with the
     main weight accumulates over taps in PSUM -> output
"""

import sys

sys.path.insert(0, "/opt/trn_rl_repo")

import numpy as np
import ml_dtypes

import concourse.bass as bass
import concourse.mybir as mybir
from concourse.bass_utils import run_bass_kernel_spmd
from concourse.tile import TileContext
from concourse.vector_clock import ScopedClock
from concourse.alu_op_type import AluOpType

F32 = mybir.dt.float32
BF16 = mybir.dt.bfloat16
I32 = mybir.dt.int32

K = 3
PAD = 1
H = W = 48
HW = H * W          # 2304
CIN = COUT = 256
NTAP = K * K        # 9
NPB = HW // 128     # 18 position blocks
TPAD = 2            # quad-table padding (pixels) on each side
TW = W + 2 * TPAD   # 52 table cols
TROWS = TW * TW     # 2704 table rows
QELEM = 4 * CIN     # 1024 bf16 per quad row (4 corners x 256 ch)
FBIAS = 16.0        # bias for float->int floor trick
CAST_HALF = 0.5     # HW float->int cast rounds to nearest even; subtract 0.5


# ---------------------------------------------------------------------------
# TileContext patches for this walrus build:
#  - it accepts at most ONE sem-wait per instruction -> hoist extras onto nops
# ---------------------------------------------------------------------------

def _make_wait_nop(nc, engine, waits):
    inst = mybir.InstNoOp(name=nc.get_next_instruction_name(), ins=[], outs=[])
    inst.engine = engine
    inst.sync_info = mybir.SyncInfo(on_wait=list(waits), on_update=[])
    nc.register_instruction(inst)
    return inst


def split_excess_waits(nc, max_waits=1):
    for _bname, bbb in nc.bb_map.items():
        bb = bbb.bb
        changed = False
        new = []
        for inst in bb.instructions:
            si = inst.sync_info
            waits = list(si.on_wait or []) if si else []
            if len(waits) > max_waits:
                keep = waits[:max_waits]
                extra = waits[max_waits:]
                for i in range(0, len(extra), max_waits):
                    new.append(_make_wait_nop(nc, inst.engine, extra[i:i + max_waits]))
                si.on_wait = keep
                changed = True
            new.append(inst)
        if changed:
            bb.instructions = new


class PatchedTC(TileContext):
    def _drain_and_barrier(self, tick_clock, wait_clock):
        drain_inst = self.nc.sync.drain()
        wait_clock.add_sem_waits(
            drain_inst.ins, ScopedClock({None: tick_clock.global_clock})
        )
        self.nc.all_engine_barrier()
        popped = self.nc._tile_sem_poison_stack.pop()
        assert popped is self._sem_poison
        self.nc.clear_and_free_semaphores(list(self.sems.allocated().values()))
        self.nc.all_engine_barrier()

    def __exit__(self, *args):
        r = super().__exit__(*args)
        split_excess_waits(self.nc)
        return r


# ---------------------------------------------------------------------------
# Kernel builder
# ---------------------------------------------------------------------------

def build_kernel(stage="full"):
    nc = bass.Bass()

    # --- DRAM parameters (per-core shards prepared by host) ---
    xpad = nc.dram_tensor("xpad", [2, 128, 50 * 50], F32, kind="ExternalInput")
    idf = nc.dram_tensor("idf", [128, 128], F32, kind="ExternalInput")
    idb = nc.dram_tensor("idb", [128, 128], BF16, kind="ExternalInput")
    xq = nc.dram_tensor("xq", [TROWS, QELEM], BF16, kind="ExternalInput")
    womT = nc.dram_tensor("womT", [NTAP, 2, 128, 27], F32, kind="ExternalInput")
    ob = nc.dram_tensor("ob", [27, 1], F32, kind="ExternalInput")
    wcT = nc.dram_tensor("wcT", [NTAP, 2, 128, 256], BF16, kind="ExternalInput")
    # BY/BX: oy + ky - 1 / ox + kx - 1 per (pblock, lane, tap), fp32
    byx = nc.dram_tensor("byx", [2, NPB, 128, NTAP], F32, kind="ExternalInput")

    if stage == "om":
        om_out = nc.dram_tensor("om_out", [NPB, 128, 27], F32, kind="ExternalOutput")
    elif stage == "idxw":
        idx_out = nc.dram_tensor("idx_out", [NPB, 128, NTAP], I32, kind="ExternalOutput")
        w_out = nc.dram_tensor("w_out", [NPB, 128, 4 * NTAP], F32, kind="ExternalOutput")
    elif stage == "vals":
        vals_out = nc.dram_tensor("vals_out", [128, NTAP, 2, HW], F32, kind="ExternalOutput")
    else:
        out = nc.dram_tensor("out", [2, 128, HW], F32, kind="ExternalOutput")

    with PatchedTC(nc) as tc:
        import contextlib
        with contextlib.ExitStack() as ctx:
            _build_body(ctx, tc, nc, stage, locals())
    return nc


def _build_body(ctx, tc, nc, stage, T):
    xpad, xq, womT, ob, wcT, byx = (
        T["xpad"], T["xq"], T["womT"], T["ob"], T["wcT"], T["byx"],
    )

    const_pool = ctx.enter_context(tc.tile_pool(name="const", bufs=1))
    sb = ctx.enter_context(tc.tile_pool(name="sb", bufs=1))
    psum = ctx.enter_context(tc.tile_pool(name="psum", bufs=2, space="PSUM"))
    psum_t = ctx.enter_context(tc.tile_pool(name="psum_t", bufs=2, space="PSUM"))
    gpool = ctx.enter_context(tc.tile_pool(name="gath", bufs=8))
    vpool = ctx.enter_context(tc.tile_pool(name="vals", bufs=1))
    cpool = ctx.enter_context(tc.tile_pool(name="contr", bufs=2, space="PSUM"))

    # --- constants / inputs to SBUF ---
    ident = const_pool.tile([128, 128], F32)
    nc.sync.dma_start(out=ident[:], in_=T["idf"][:])
    ident_b = const_pool.tile([128, 128], BF16)
    nc.sync.dma_start(out=ident_b[:], in_=T["idb"][:])

    xp_sb = const_pool.tile([128, 2, 50 * 50], F32, tag="xp")
    for cb in range(2):
        nc.sync.dma_start(out=xp_sb[:, cb], in_=xpad[cb])

    wom_sb = const_pool.tile([128, NTAP, 2, 27], F32)
    nc.sync.dma_start(out=wom_sb[:], in_=womT[:].rearrange("t c p m -> p t c m"))

    ob_sb = const_pool.tile([27, 1], F32)
    nc.sync.dma_start(out=ob_sb[:], in_=ob[:])

    wc_sb = const_pool.tile([128, NTAP, 2, 256], BF16)
    nc.sync.dma_start(out=wc_sb[:], in_=wcT[:].rearrange("t c p m -> p t c m"))

    byx_sb = const_pool.tile([128, 2, NPB, NTAP], F32)
    nc.sync.dma_start(out=byx_sb[:], in_=byx[:].rearrange("a b p t -> p a b t"))

    # =====================================================================
    # Stage 1: offset/mask conv -> om_sb [27, HW] fp32
    # =====================================================================
    om_sb = sb.tile([27, HW], F32)
    # row groups of 10 output rows (N=480 <= 512 fp32 psum limit)
    row_groups = [(0, 10), (10, 10), (20, 10), (30, 10), (40, 8)]
    for (r0, nr) in row_groups:
        ps = psum.tile([27, 480], F32, tag="omps")
        n = nr * W
        first = True
        for tap in range(NTAP):
            dy, dx = tap // K, tap % K
            rhs = xp_sb[:, 0].rearrange("p (h w) -> p h w", h=50)[
                :, dy + r0:dy + r0 + nr, dx:dx + W
            ]
            rhs2 = xp_sb[:, 1].rearrange("p (h w) -> p h w", h=50)[
                :, dy + r0:dy + r0 + nr, dx:dx + W
            ]
            nc.tensor.matmul(ps[:, :n], lhsT=wom_sb[:, tap, 0], rhs=rhs,
                             start=first, stop=False)
            first = False
            last = tap == NTAP - 1
            nc.tensor.matmul(ps[:, :n], lhsT=wom_sb[:, tap, 1], rhs=rhs2,
                             start=False, stop=last)
        # bias add during PSUM -> SBUF move (ACT, per-partition bias)
        nc.scalar.activation(
            out=om_sb[:, r0 * W:r0 * W + n], in_=ps[:, :n],
            func=mybir.ActivationFunctionType.Identity,
            bias=ob_sb[:], scale=1.0,
        )

    # transpose to position-major: om_t [128, NPB, 27]
    om_t = sb.tile([128, NPB, 27], F32)
    for pb in range(NPB):
        pst = psum_t.tile([128, 27], F32, tag="omT")
        nc.tensor.transpose(out=pst[:], in_=om_sb[:, pb * 128:(pb + 1) * 128],
                            identity=ident[:27, :27])
        nc.vector.tensor_copy(out=om_t[:, pb], in_=pst[:])

    if stage == "om":
        nc.sync.dma_start(out=T["om_out"][:].rearrange("b p m -> p b m"), in_=om_t[:])
        return

    # =====================================================================
    # Stage 2: index + corner weight computation (batched over all pblocks)
    #   om_t channels: 2t = dy_t, 2t+1 = dx_t (t in 0..8), 18..26 = mask
    # =====================================================================
    # strided AP views [128, NPB, 9]
    off_y = om_t[:, :, 0:18:2]
    off_x = om_t[:, :, 1:18:2]
    msk_l = om_t[:, :, 18:27]

    shp = [128, NPB, NTAP]
    py = sb.tile(shp, F32)
    px = sb.tile(shp, F32)
    msk = sb.tile(shp, F32)
    nc.scalar.activation(out=msk[:], in_=msk_l,
                         func=mybir.ActivationFunctionType.Sigmoid)

    # py = off_y + (oy + ky - 1) ; px = off_x + (ox + kx - 1)
    nc.vector.tensor_tensor(out=py[:], in0=off_y, in1=byx_sb[:, 0], op=AluOpType.add)
    nc.vector.tensor_tensor(out=px[:], in0=off_x, in1=byx_sb[:, 1], op=AluOpType.add)

    # floor via biased round-to-nearest-even cast: iy = int(py + FBIAS - 0.5)
    ty = sb.tile(shp, F32)
    tx = sb.tile(shp, F32)
    nc.vector.tensor_scalar(out=ty[:], in0=py[:], scalar1=FBIAS - CAST_HALF,
                            scalar2=None, op0=AluOpType.add)
    nc.vector.tensor_scalar(out=tx[:], in0=px[:], scalar1=FBIAS - CAST_HALF,
                            scalar2=None, op0=AluOpType.add)
    iy = sb.tile(shp, I32)
    ix = sb.tile(shp, I32)
    nc.vector.tensor_copy(out=iy[:], in_=ty[:])
    nc.vector.tensor_copy(out=ix[:], in_=tx[:])
    # back to float for fractional part: fy = py + FBIAS - iyf
    iyf = sb.tile(shp, F32)
    ixf = sb.tile(shp, F32)
    nc.vector.tensor_copy(out=iyf[:], in_=iy[:])
    nc.vector.tensor_copy(out=ixf[:], in_=ix[:])
    fy = sb.tile(shp, F32)
    fx = sb.tile(shp, F32)
    nc.vector.tensor_scalar(out=py[:], in0=py[:], scalar1=FBIAS, scalar2=None,
                            op0=AluOpType.add)
    nc.vector.tensor_scalar(out=px[:], in0=px[:], scalar1=FBIAS, scalar2=None,
                            op0=AluOpType.add)
    nc.vector.tensor_tensor(out=fy[:], in0=py[:], in1=iyf[:], op=AluOpType.subtract)
    nc.vector.tensor_tensor(out=fx[:], in0=px[:], in1=ixf[:], op=AluOpType.subtract)

    # table index: q = clamp(iy - FBIAS + TPAD, 0, TW-1)*TW + clamp(ix - FBIAS + TPAD, 0, TW-1)
    qy = sb.tile(shp, I32)
    qx = sb.tile(shp, I32)
    nc.vector.tensor_scalar(out=qy[:], in0=iy[:], scalar1=int(-FBIAS) + TPAD,
                            scalar2=0, op0=AluOpType.add, op1=AluOpType.max)
    nc.vector.tensor_scalar(out=qy[:], in0=qy[:], scalar1=TW - 1, scalar2=TW,
                            op0=AluOpType.min, op1=AluOpType.mult)
    nc.vector.tensor_scalar(out=qx[:], in0=ix[:], scalar1=int(-FBIAS) + TPAD,
                            scalar2=0, op0=AluOpType.add, op1=AluOpType.max)
    nc.vector.tensor_scalar(out=qx[:], in0=qx[:], scalar1=TW - 1, scalar2=None,
                            op0=AluOpType.min)
    qidx = sb.tile(shp, I32)
    nc.vector.tensor_tensor(out=qidx[:], in0=qy[:], in1=qx[:], op=AluOpType.add)

    # corner weights (with mask folded in): A=(1-fy)(1-fx)m B=(1-fy)fx m
    #                                       C=fy(1-fx)m    D=fy fx m
    gy = sb.tile(shp, F32)
    gx = sb.tile(shp, F32)
    nc.vector.tensor_scalar(out=gy[:], in0=fy[:], scalar1=-1.0, scalar2=1.0,
                            op0=AluOpType.mult, op1=AluOpType.add)
    nc.vector.tensor_scalar(out=gx[:], in0=fx[:], scalar1=-1.0, scalar2=1.0,
                            op0=AluOpType.mult, op1=AluOpType.add)
    u0 = sb.tile(shp, F32)
    u1 = sb.tile(shp, F32)
    nc.vector.tensor_tensor(out=u0[:], in0=gy[:], in1=msk[:], op=AluOpType.mult)
    nc.vector.tensor_tensor(out=u1[:], in0=fy[:], in1=msk[:], op=AluOpType.mult)
    # weights tile [128, NPB, 4, NTAP] bf16 (corner-major for easy slicing)
    wq = sb.tile([128, NPB, 4, NTAP], F32)
    nc.vector.tensor_tensor(out=wq[:, :, 0], in0=u0[:], in1=gx[:], op=AluOpType.mult)
    nc.vector.tensor_tensor(out=wq[:, :, 1], in0=u0[:], in1=fx[:], op=AluOpType.mult)
    nc.vector.tensor_tensor(out=wq[:, :, 2], in0=u1[:], in1=gx[:], op=AluOpType.mult)
    nc.vector.tensor_tensor(out=wq[:, :, 3], in0=u1[:], in1=fx[:], op=AluOpType.mult)

    if stage == "idxw":
        nc.sync.dma_start(out=T["idx_out"][:].rearrange("b p t -> p b t"), in_=qidx[:])
        nc.sync.dma_start(out=T["w_out"][:].rearrange("b p m -> p b m"),
                          in_=wq[:].rearrange("p b c t -> p b (c t)"))
        return

    # =====================================================================
    # Stage 3+4: gathers + corner combine -> vals (bf16) per (tap, pblock)
    # Stage 5: transpose vals to channel-major + contraction matmul
    # =====================================================================
    valsT = vpool.tile([128, NTAP, 2, HW], BF16)  # [c-part, tap, cblk, p]

    for pb in range(NPB):
        for tap in range(NTAP):
            g = gpool.tile([128, QELEM], BF16, tag="g")
            nc.gpsimd.indirect_dma_start(
                out=g[:], out_offset=None, in_=xq[:],
                in_offset=bass.IndirectOffsetOnAxis(ap=qidx[:, pb, tap:tap + 1], axis=0),
            )
            v = gpool.tile([128, 256], BF16, tag="v")
            nc.vector.tensor_scalar(
                out=v[:], in0=g[:, 0:256], scalar1=wq[:, pb, 0, tap:tap + 1],
                scalar2=None, op0=AluOpType.mult)
            for c in range(1, 4):
                nc.vector.scalar_tensor_tensor(
                    out=v[:], in0=g[:, 256 * c:256 * (c + 1)],
                    scalar=wq[:, pb, c, tap:tap + 1], in1=v[:],
                    op0=AluOpType.mult, op1=AluOpType.add)
            # transpose [128p, 256c] -> two [128c, 128p] PSUM tiles -> valsT
            for cb in range(2):
                pt = psum_t.tile([128, 128], BF16, tag="vT")
                nc.tensor.transpose(out=pt[:], in_=v[:, 128 * cb:128 * (cb + 1)],
                                    identity=ident_b[:])
                nc.scalar.activation(
                    out=valsT[:, tap, cb, pb * 128:(pb + 1) * 128], in_=pt[:],
                    func=mybir.ActivationFunctionType.Copy)

    if stage == "vals":
        # dump raw valsT layout [c(128), tap, cblk, p(HW)] cast to f32
        vv = sb.tile([128, NTAP, 2, HW], F32)
        nc.vector.tensor_copy(out=vv[:], in_=valsT[:])
        nc.sync.dma_start(out=T["vals_out"][:], in_=vv[:])
        return

    # contraction: out[o, p] = sum_{tap, cblk} wcT[tap,cblk].T @ valsT[tap,cblk]
    out_sb = sb.tile([128, 2, HW], F32)
    CH = 512  # psum free-dim chunk
    for p0 in range(0, HW, CH):
        n = min(CH, HW - p0)
        for ob_i in range(2):
            ps = cpool.tile([128, CH], F32, tag="ops")
            first = True
            for tap in range(NTAP):
                for cb in range(2):
                    nc.tensor.matmul(
                        ps[:, :n],
                        lhsT=wc_sb[:, tap, cb, 128 * ob_i:128 * (ob_i + 1)],
                        rhs=valsT[:, tap, cb, p0:p0 + n],
                        start=first, stop=(tap == NTAP - 1 and cb == 1),
                    )
                    first = False
            nc.vector.tensor_copy(out=out_sb[:, ob_i, p0:p0 + n], in_=ps[:, :n])

    nc.sync.dma_start(out=T["out"][:].rearrange("b p m -> p b m"), in_=out_sb[:])


# ---------------------------------------------------------------------------
# Host-side wrapper
# ---------------------------------------------------------------------------

def _prep_core_inputs(xb, weight, off_w, off_b, mask_w, mask_b):
    """Build per-core input dict for one image xb [256, 48, 48] fp32."""
    xpad = np.zeros((256, 50, 50), np.float32)
    xpad[:, 1:49, 1:49] = xb
    xpad2 = xpad.reshape(2, 128, 50 * 50)

    # quad table: rows keyed (y+TPAD)*TW + (x+TPAD), y,x in [-TPAD, 48+TPAD)
    xb_bf = xb.astype(ml_dtypes.bfloat16)
    ext = np.zeros((256, TW + 1, TW + 1), ml_dtypes.bfloat16)
    ext[:, TPAD:TPAD + 48, TPAD:TPAD + 48] = xb_bf
    # corners: (y,x), (y,x+1), (y+1,x), (y+1,x+1)
    q = np.zeros((TW, TW, 4, 256), ml_dtypes.bfloat16)
    q[:, :, 0] = ext[:, :TW, :TW].transpose(1, 2, 0)
    q[:, :, 1] = ext[:, :TW, 1:TW + 1].transpose(1, 2, 0)
    q[:, :, 2] = ext[:, 1:TW + 1, :TW].transpose(1, 2, 0)
    q[:, :, 3] = ext[:, 1:TW + 1, 1:TW + 1].transpose(1, 2, 0)
    xq = q.reshape(TROWS, QELEM)

    # offset+mask conv weights -> lhsT blocks [tap, cblk, 128c, 27]
    wom = np.concatenate([off_w, mask_w], axis=0)          # [27, 256, 3, 3]
    womT = wom.reshape(27, 2, 128, K * K).transpose(3, 1, 2, 0).copy()
    ob = np.concatenate([off_b, mask_b])[:, None].astype(np.float32)

    # main weight -> lhsT blocks [tap, cblk, 128c, 256o] bf16
    wcT = weight.reshape(256, 2, 128, K * K).transpose(3, 1, 2, 0).astype(
        ml_dtypes.bfloat16).copy()

    # BY/BX [2, NPB, 128, 9]
    p = np.arange(HW)
    oy, ox = p // W, p % W
    ky, kx = np.meshgrid(np.arange(K), np.arange(K), indexing="ij")
    BY = (oy[:, None] + ky.reshape(-1)[None, :] - 1).astype(np.float32)
    BX = (ox[:, None] + kx.reshape(-1)[None, :] - 1).astype(np.float32)
    byx = np.stack([BY, BX]).reshape(2, NPB, 128, NTAP)

    idf = np.eye(128, dtype=np.float32)
    idb = np.eye(128, dtype=np.float32).astype(ml_dtypes.bfloat16)

    return dict(xpad=xpad2, xq=xq, womT=womT, ob=ob, wcT=wcT, byx=byx,
                idf=idf, idb=idb)


_CACHED = {}


def kernel(x, weight, off_w, off_b, mask_w, mask_b, _stage="full", _trace=False):
    x = np.asarray(x, np.float32)
    weight = np.asarray(weight, np.float32)
    off_w = np.asarray(off_w, np.float32)
    off_b = np.asarray(off_b, np.float32)
    mask_w = np.asarray(mask_w, np.float32)
    mask_b = np.asarray(mask_b, np.float32)
    B = x.shape[0]
    assert B == 8

    if _stage not in _CACHED:
        _CACHED[_stage] = build_kernel(_stage)
    nc = _CACHED[_stage]

    in_maps = [
        _prep_core_inputs(x[b], weight, off_w, off_b, mask_w, mask_b)
        for b in range(B)
    ]
    res = run_bass_kernel_spmd(nc, in_maps, core_ids=list(range(8)), trace=_trace)
    if _stage != "full":
        return res

    out = np.empty((B, COUT, H, W), np.float32)
    for b in range(B):
        o = res.results[b]["out"]           # [2, 128, HW]
        out[b] = o.reshape(COUT, H, W)
    kernel._last_exec_time_ns = res.exec_time_ns
    return out



# revision 18
# speedup vs baseline: 1.1077x; 1.1077x over previous
"""Deformable Conv2d (modulated, torchvision v2 layout) on 8 Trainium2 NeuronCores.

Strategy: data-parallel over batch (B=8 -> 1 image per core).
Per core, entirely on device:
  1. offset/mask convs on TensorE (im2col-by-shifted-AP matmuls, bf16)
  2. transpose conv output to position-major [p, 27]; compute bilinear
     corner weights (x mask, x validity) and int32 quad-table indices on
     VectorE with batched strided APs
  3. 18 batched indirect DMAs (one per 128-position block, 9 taps each)
     gather 2KB bf16 "quad" rows (all 4 bilinear corners of one
     tap/position, pre-packed host-side with 2-pixel zero padding so
     out-of-bounds sampling is exactly zero like the reference)
  4. corner combine: 4 per-partition-scalar products (tensor_scalar on
     DVE at 4x mode, activation-scale on ACT) + batched tensor_tensor
     adds (DVE/GpSimd)
  5. TensorE transposes vals to channel-major; contraction matmul with
     the main weight accumulates over taps in PSUM -> output, chunked
     per 256 positions and overlapped with the gather pipeline
"""

import sys

sys.path.insert(0, "/opt/trn_rl_repo")

import numpy as np
import ml_dtypes

import concourse.bass as bass
import concourse.mybir as mybir
from concourse.bass_utils import run_bass_kernel_spmd
from concourse.tile import TileContext
from concourse.vector_clock import ScopedClock
from concourse.alu_op_type import AluOpType
from concourse.library_config import mlp

F32 = mybir.dt.float32
BF16 = mybir.dt.bfloat16
I32 = mybir.dt.int32
I16 = mybir.dt.int16

K = 3
PAD = 1
H = W = 48
HW = H * W          # 2304
CIN = COUT = 256
NTAP = K * K        # 9
NPB = HW // 128     # 18 position blocks
TPAD = 2            # quad-table padding (pixels) on each side
TW = W + 2 * TPAD   # 52 table cols
TROWS = TW * TW     # 2704 table rows
QELEM = 4 * CIN     # 1024 bf16 per quad row (4 corners x 256 ch)
FBIAS = 16.0        # bias for float->int floor trick
CAST_HALF = 0.5     # HW float->int cast rounds to nearest even; subtract 0.5

NRG = 6             # om-conv row groups (8 rows each, = 3 pblocks)
RGW = 8 * W         # columns per row group (384)

POOL_ADD = False    # do one corner-sum per pblock on GpSimd (else VectorE)
GATHER_MODE = "indirect"  # "ant" (dma_gather, 1/pblock) | "indirect" (1/tap)

Copy = mybir.ActivationFunctionType.Copy
Ident = mybir.ActivationFunctionType.Identity
Sigmoid = mybir.ActivationFunctionType.Sigmoid


# ---------------------------------------------------------------------------
# TileContext patches for this walrus build:
#  - it accepts at most ONE sem-wait per instruction -> hoist extras onto nops
# ---------------------------------------------------------------------------

def _make_wait_nop(nc, engine, waits):
    inst = mybir.InstNoOp(name=nc.get_next_instruction_name(), ins=[], outs=[])
    inst.engine = engine
    inst.sync_info = mybir.SyncInfo(on_wait=list(waits), on_update=[])
    nc.register_instruction(inst)
    return inst


def split_excess_waits(nc, max_waits=1):
    for _bname, bbb in nc.bb_map.items():
        bb = bbb.bb
        changed = False
        new = []
        for inst in bb.instructions:
            si = inst.sync_info
            waits = list(si.on_wait or []) if si else []
            if len(waits) > max_waits:
                keep = waits[:max_waits]
                extra = waits[max_waits:]
                for i in range(0, len(extra), max_waits):
                    new.append(_make_wait_nop(nc, inst.engine, extra[i:i + max_waits]))
                si.on_wait = keep
                changed = True
            new.append(inst)
        if changed:
            bb.instructions = new


class PatchedTC(TileContext):
    def _drain_and_barrier(self, tick_clock, wait_clock):
        drain_inst = self.nc.sync.drain()
        wait_clock.add_sem_waits(
            drain_inst.ins, ScopedClock({None: tick_clock.global_clock})
        )
        self.nc.all_engine_barrier()
        popped = self.nc._tile_sem_poison_stack.pop()
        assert popped is self._sem_poison
        self.nc.clear_and_free_semaphores(list(self.sems.allocated().values()))
        self.nc.all_engine_barrier()

    def __exit__(self, *args):
        r = super().__exit__(*args)
        split_excess_waits(self.nc)
        return r


# ---------------------------------------------------------------------------
# Kernel builder
# ---------------------------------------------------------------------------

def build_kernel(stage="full"):
    nc = bass.Bass()

    # --- DRAM parameters (per-core shards prepared by host) ---
    xpad = nc.dram_tensor("xpad", [2, 128, 50 * 50], BF16, kind="ExternalInput")
    idb = nc.dram_tensor("idb", [128, 128], BF16, kind="ExternalInput")
    xq = nc.dram_tensor("xq", [TROWS, QELEM], BF16, kind="ExternalInput")
    womT = nc.dram_tensor("womT", [128, NTAP, 2, 27], BF16, kind="ExternalInput")
    ob = nc.dram_tensor("ob", [27, 1], F32, kind="ExternalInput")
    wcT = nc.dram_tensor("wcT", [128, NTAP, 2, 256], BF16, kind="ExternalInput")
    # BY/BX: oy + ky - 1 / ox + kx - 1 per (lane, axis, pblock, tap), fp32
    byx = nc.dram_tensor("byx", [128, 2, NPB, NTAP], F32, kind="ExternalInput")

    if GATHER_MODE == "ant":
        idxs_dram = nc.dram_tensor("idxs_scratch", [128, NPB, NTAP], I16,
                                   kind="Internal")

    if stage == "om":
        om_out = nc.dram_tensor("om_out", [NPB, 128, 27], F32, kind="ExternalOutput")
    elif stage == "idxw":
        idx_out = nc.dram_tensor("idx_out", [NPB, 128, NTAP], I32, kind="ExternalOutput")
        w_out = nc.dram_tensor("w_out", [NPB, 128, 4 * NTAP], F32, kind="ExternalOutput")
    else:
        out = nc.dram_tensor("out", [2, 128, HW], F32, kind="ExternalOutput")

    with PatchedTC(nc) as tc:
        import contextlib
        with contextlib.ExitStack() as ctx:
            _build_body(ctx, tc, nc, stage, locals())
    if GATHER_MODE == "ant":
        # raw Bass skips Bacc's codegen_inst_isa_subclasses pass; without it
        # walrus sees empty .instr for extended-ISA insts -> "ISA wrong length"
        from concourse.library_overlay import lower_extended_insts
        lower_extended_insts(nc)
    return nc


def _build_body(ctx, tc, nc, stage, T):
    xpad, xq, womT, ob, wcT, byx = (
        T["xpad"], T["xq"], T["womT"], T["ob"], T["wcT"], T["byx"],
    )

    const_pool = ctx.enter_context(tc.tile_pool(name="const", bufs=1))
    sb = ctx.enter_context(tc.tile_pool(name="sb", bufs=1))
    gpool = ctx.enter_context(tc.tile_pool(name="gath", bufs=3))
    ppool = ctx.enter_context(tc.tile_pool(name="prod", bufs=2))
    vpool = ctx.enter_context(tc.tile_pool(name="vals", bufs=2))
    opool = ctx.enter_context(tc.tile_pool(name="oc", bufs=2))
    psum_om = ctx.enter_context(tc.tile_pool(name="psom", bufs=2, space="PSUM"))
    psum_omt = ctx.enter_context(tc.tile_pool(name="psomt", bufs=1, space="PSUM"))
    psum_t = ctx.enter_context(tc.tile_pool(name="pst", bufs=3, space="PSUM"))
    psum_c = ctx.enter_context(tc.tile_pool(name="psc", bufs=2, space="PSUM"))

    # --- constants / inputs to SBUF (all contiguous partition-major) ---
    ident_b = const_pool.tile([128, 128], BF16)
    nc.sync.dma_start(out=ident_b[:], in_=T["idb"][:])

    xp_sb = const_pool.tile([128, 2, 50 * 50], BF16, tag="xp")
    for cb in range(2):
        nc.sync.dma_start(out=xp_sb[:, cb], in_=xpad[cb])

    wom_sb = const_pool.tile([128, NTAP, 2, 27], BF16)
    nc.sync.dma_start(out=wom_sb[:], in_=womT[:])

    ob_sb = const_pool.tile([27, 1], F32)
    nc.sync.dma_start(out=ob_sb[:], in_=ob[:])

    wc_sb = const_pool.tile([128, NTAP, 2, 256], BF16)
    nc.sync.dma_start(out=wc_sb[:], in_=wcT[:])

    byx_sb = const_pool.tile([128, 2, NPB, NTAP], F32)
    nc.sync.dma_start(out=byx_sb[:], in_=byx[:])

    # =====================================================================
    # Stage 1: offset/mask conv -> om_sb [27, HW] bf16 (6 rowgroups x 8 rows)
    # =====================================================================
    om_sb = sb.tile([27, HW], BF16)
    xp_v = xp_sb[:].rearrange("p c (h w) -> p c h w", h=50)
    for g in range(NRG):
        ps = psum_om.tile([27, RGW], F32, tag="omps")
        r0 = 8 * g
        first = True
        for tap in range(NTAP):
            dy, dx = tap // K, tap % K
            for cb in range(2):
                nc.tensor.matmul(
                    ps[:], lhsT=wom_sb[:, tap, cb],
                    rhs=xp_v[:, cb, dy + r0:dy + r0 + 8, dx:dx + W],
                    start=first, stop=(tap == NTAP - 1 and cb == 1),
                )
                first = False
        # bias add during PSUM -> SBUF move (ACT, per-partition bias)
        nc.scalar.activation(
            out=om_sb[:, g * RGW:(g + 1) * RGW], in_=ps[:],
            func=Ident, bias=ob_sb[:], scale=1.0,
        )

    # transpose to position-major: om_t [128, NPB, 27] fp32
    om_t = sb.tile([128, NPB, 27], F32)
    for pb in range(NPB):
        pst = psum_omt.tile([128, 27], BF16, tag="omT")
        nc.tensor.transpose(out=pst[:], in_=om_sb[:, pb * 128:(pb + 1) * 128],
                            identity=ident_b[:27, :27])
        nc.scalar.activation(out=om_t[:, pb], in_=pst[:], func=Copy)

    if stage == "om":
        nc.sync.dma_start(out=T["om_out"][:].rearrange("b p m -> p b m"), in_=om_t[:])
        return

    # =====================================================================
    # Stage 2: index + corner weight computation (two halves of 9 pblocks)
    #   om_t channels: 2t = dy_t, 2t+1 = dx_t (t in 0..8), 18..26 = mask
    # =====================================================================
    shp = [128, NPB, NTAP]
    py = sb.tile(shp, F32)
    px = sb.tile(shp, F32)
    msk = sb.tile(shp, F32)
    ty = sb.tile(shp, F32)
    tx = sb.tile(shp, F32)
    iy = sb.tile(shp, I32)
    ix = sb.tile(shp, I32)
    iyf = sb.tile(shp, F32)
    ixf = sb.tile(shp, F32)
    fy = sb.tile(shp, F32)
    fx = sb.tile(shp, F32)
    qy = sb.tile(shp, I32)
    qx = sb.tile(shp, I32)
    qidx = sb.tile(shp, I32)
    gy = sb.tile(shp, F32)
    gx = sb.tile(shp, F32)
    u0 = sb.tile(shp, F32)
    u1 = sb.tile(shp, F32)
    wq = sb.tile([128, NPB, 4, NTAP], F32)

    for h in range(2):
        s = slice(9 * h, 9 * h + 9)
        off_y = om_t[:, s, 0:18:2]
        off_x = om_t[:, s, 1:18:2]
        nc.scalar.activation(out=msk[:, s], in_=om_t[:, s, 18:27], func=Sigmoid)
        nc.vector.tensor_tensor(out=py[:, s], in0=off_y, in1=byx_sb[:, 0, s],
                                op=AluOpType.add)
        nc.vector.tensor_tensor(out=px[:, s], in0=off_x, in1=byx_sb[:, 1, s],
                                op=AluOpType.add)
        # floor via biased round-to-nearest-even cast: iy = int(py + 15.5)
        nc.vector.tensor_scalar(out=ty[:, s], in0=py[:, s],
                                scalar1=FBIAS - CAST_HALF, scalar2=None,
                                op0=AluOpType.add)
        nc.vector.tensor_scalar(out=tx[:, s], in0=px[:, s],
                                scalar1=FBIAS - CAST_HALF, scalar2=None,
                                op0=AluOpType.add)
        nc.vector.tensor_copy(out=iy[:, s], in_=ty[:, s])
        nc.vector.tensor_copy(out=ix[:, s], in_=tx[:, s])
        nc.vector.tensor_copy(out=iyf[:, s], in_=iy[:, s])
        nc.vector.tensor_copy(out=ixf[:, s], in_=ix[:, s])
        # fractional parts: fy = (py + FBIAS) - iyf
        nc.vector.scalar_tensor_tensor(out=fy[:, s], in0=py[:, s], scalar=FBIAS,
                                       in1=iyf[:, s], op0=AluOpType.add,
                                       op1=AluOpType.subtract)
        nc.vector.scalar_tensor_tensor(out=fx[:, s], in0=px[:, s], scalar=FBIAS,
                                       in1=ixf[:, s], op0=AluOpType.add,
                                       op1=AluOpType.subtract)
        # table index: q = clamp(iy-16+2, 0, 51)*52 + clamp(ix-16+2, 0, 51)
        nc.vector.tensor_scalar(out=qy[:, s], in0=iy[:, s],
                                scalar1=int(-FBIAS) + TPAD, scalar2=0,
                                op0=AluOpType.add, op1=AluOpType.max)
        nc.vector.tensor_scalar(out=qy[:, s], in0=qy[:, s], scalar1=TW - 1,
                                scalar2=TW, op0=AluOpType.min,
                                op1=AluOpType.mult)
        nc.vector.tensor_scalar(out=qx[:, s], in0=ix[:, s],
                                scalar1=int(-FBIAS) + TPAD, scalar2=0,
                                op0=AluOpType.add, op1=AluOpType.max)
        nc.vector.tensor_scalar(out=qx[:, s], in0=qx[:, s], scalar1=TW - 1,
                                scalar2=None, op0=AluOpType.min)
        nc.vector.tensor_tensor(out=qidx[:, s], in0=qy[:, s], in1=qx[:, s],
                                op=AluOpType.add)
        # corner weights (with mask folded in): A=(1-fy)(1-fx)m B=(1-fy)fx m
        #                                       C=fy(1-fx)m    D=fy fx m
        nc.vector.tensor_scalar(out=gy[:, s], in0=fy[:, s], scalar1=-1.0,
                                scalar2=1.0, op0=AluOpType.mult,
                                op1=AluOpType.add)
        nc.vector.tensor_scalar(out=gx[:, s], in0=fx[:, s], scalar1=-1.0,
                                scalar2=1.0, op0=AluOpType.mult,
                                op1=AluOpType.add)
        nc.vector.tensor_tensor(out=u0[:, s], in0=gy[:, s], in1=msk[:, s],
                                op=AluOpType.mult)
        nc.vector.tensor_tensor(out=u1[:, s], in0=fy[:, s], in1=msk[:, s],
                                op=AluOpType.mult)
        nc.vector.tensor_tensor(out=wq[:, s, 0], in0=u0[:, s], in1=gx[:, s],
                                op=AluOpType.mult)
        nc.vector.tensor_tensor(out=wq[:, s, 1], in0=u0[:, s], in1=fx[:, s],
                                op=AluOpType.mult)
        nc.vector.tensor_tensor(out=wq[:, s, 2], in0=u1[:, s], in1=gx[:, s],
                                op=AluOpType.mult)
        nc.vector.tensor_tensor(out=wq[:, s, 3], in0=u1[:, s], in1=fx[:, s],
                                op=AluOpType.mult)

    if stage == "idxw":
        nc.sync.dma_start(out=T["idx_out"][:].rearrange("b p t -> p b t"), in_=qidx[:])
        nc.sync.dma_start(out=T["w_out"][:].rearrange("b p m -> p b m"),
                          in_=wq[:].rearrange("p b c t -> p b (c t)"))
        return

    # =====================================================================
    # Stage 3-5: gather + combine + transpose per pblock; contraction per
    # 2 pblocks (256 output positions)
    # =====================================================================
    if GATHER_MODE == "ant":
        # dma_gather wants int16 indices "wrapped in 16 partitions": index
        # i = t*128 + p lives at [i % 16, i // 16] of a [128, 72]-shaped AP
        # (only partitions 0-15 carry data), i.e. wrapped[p % 16, pb, t*8 +
        # p//16] = qidx[p, pb, t]. Roundtrip through DRAM: contiguous store,
        # then a permuted-AP load (2-byte runs, HWDGE, one-time).
        nc.gpsimd.load_library(mlp)
        qidx16 = sb.tile([128, NPB, NTAP], I16)
        nc.vector.tensor_copy(out=qidx16[:], in_=qidx[:])
        nc.sync.dma_start(out=T["idxs_dram"][:], in_=qidx16[:])
        idx16_sb = sb.tile([128, NPB, 8 * NTAP], I16)
        # gather views the idxs as [128, 72] but only reads partitions 0-15;
        # zero the rest so the sim's uninitialized-read check passes
        nc.vector.memset(idx16_sb[:], 0)
        nc.sync.dma_start(
            out=idx16_sb[0:16].rearrange("r b (t u) -> r b t u", u=8),
            in_=T["idxs_dram"][:].rearrange("(u r) b t -> r b t u", u=8))

    vals2 = None
    for pb in range(NPB):
        hlf = pb % 2
        # --- batched gather: all 9 taps of this pblock in one instruction
        g = gpool.tile([128, NTAP, QELEM], BF16, tag="g")
        if GATHER_MODE == "ant":
            nc.gpsimd.dma_gather(
                out_ap=g[:], in_ap=xq[:], idxs_ap=idx16_sb[:, pb],
                num_idxs=NTAP * 128, num_idxs_reg=NTAP * 128,
                elem_size=QELEM)
        else:
            for t in range(NTAP):
                nc.gpsimd.indirect_dma_start(
                    out=g[:, t], out_offset=None, in_=xq[:],
                    in_offset=bass.IndirectOffsetOnAxis(
                        ap=qidx[:, pb, t:t + 1], axis=0),
                )
        # --- corner products (per-partition scalars, FD=256)
        p0 = ppool.tile([128, NTAP, 256], BF16, tag="p0")
        p1 = ppool.tile([128, NTAP, 256], BF16, tag="p1")
        p2 = ppool.tile([128, NTAP, 256], BF16, tag="p2")
        p3 = ppool.tile([128, NTAP, 256], BF16, tag="p3")
        v = ppool.tile([128, NTAP, 256], BF16, tag="v")
        for t in range(NTAP):
            nc.vector.tensor_scalar(out=p0[:, t], in0=g[:, t, 0:256],
                                    scalar1=wq[:, pb, 0, t:t + 1], scalar2=None,
                                    op0=AluOpType.mult)
            nc.vector.tensor_scalar(out=p1[:, t], in0=g[:, t, 256:512],
                                    scalar1=wq[:, pb, 1, t:t + 1], scalar2=None,
                                    op0=AluOpType.mult)
            nc.scalar.activation(out=p2[:, t], in_=g[:, t, 512:768], func=Copy,
                                 bias=0.0, scale=wq[:, pb, 2, t:t + 1])
            nc.scalar.activation(out=p3[:, t], in_=g[:, t, 768:1024], func=Copy,
                                 bias=0.0, scale=wq[:, pb, 3, t:t + 1])
        # --- corner sums (batched FD=2304)
        nc.vector.tensor_tensor(out=p0[:], in0=p0[:], in1=p1[:], op=AluOpType.add)
        eng_add = nc.gpsimd if POOL_ADD else nc.vector
        eng_add.tensor_tensor(out=p2[:], in0=p2[:], in1=p3[:], op=AluOpType.add)
        nc.vector.tensor_tensor(out=v[:], in0=p0[:], in1=p2[:], op=AluOpType.add)

        # --- transpose to channel-major [c, tap, cb, half, pos]
        if hlf == 0:
            vals2 = vpool.tile([128, NTAP, 2, 2, 128], BF16, tag="v2")
        for k in range(3):
            pv = psum_t.tile([128, 3, 2, 128], BF16, tag="vT")
            for j in range(3):
                t = 3 * k + j
                for cb in range(2):
                    nc.tensor.transpose(out=pv[:, j, cb],
                                        in_=v[:, t, 128 * cb:128 * (cb + 1)],
                                        identity=ident_b[:])
            nc.scalar.activation(out=vals2[:, 3 * k:3 * k + 3, :, hlf, :],
                                 in_=pv[:], func=Copy)

        # --- contraction for this pair of pblocks
        if hlf == 1:
            ch = pb // 2
            ps = psum_c.tile([128, 2, 256], F32, tag="ops")
            for ob_i in range(2):
                first = True
                for t in range(NTAP):
                    for cb in range(2):
                        nc.tensor.matmul(
                            ps[:, ob_i],
                            lhsT=wc_sb[:, t, cb, 128 * ob_i:128 * (ob_i + 1)],
                            rhs=vals2[:, t, cb],
                            start=first, stop=(t == NTAP - 1 and cb == 1),
                        )
                        first = False
            oc = opool.tile([128, 2, 256], F32, tag="oc")
            nc.scalar.activation(out=oc[:], in_=ps[:], func=Copy)
            nc.sync.dma_start(
                out=T["out"][:, :, 256 * ch:256 * (ch + 1)].rearrange("b p m -> p b m"),
                in_=oc[:])


# ---------------------------------------------------------------------------
# Host-side wrapper
# ---------------------------------------------------------------------------

def _prep_core_inputs(xb, weight, off_w, off_b, mask_w, mask_b):
    """Build per-core input dict for one image xb [256, 48, 48] fp32."""
    xpad = np.zeros((256, 50, 50), np.float32)
    xpad[:, 1:49, 1:49] = xb
    xpad_bf = xpad.astype(ml_dtypes.bfloat16).reshape(2, 128, 50 * 50)

    # quad table: rows keyed (y+TPAD)*TW + (x+TPAD), y,x in [-TPAD, 48+TPAD)
    xb_bf = xb.astype(ml_dtypes.bfloat16)
    ext = np.zeros((256, TW + 1, TW + 1), ml_dtypes.bfloat16)
    ext[:, TPAD:TPAD + 48, TPAD:TPAD + 48] = xb_bf
    # corners: (y,x), (y,x+1), (y+1,x), (y+1,x+1)
    q = np.zeros((TW, TW, 4, 256), ml_dtypes.bfloat16)
    q[:, :, 0] = ext[:, :TW, :TW].transpose(1, 2, 0)
    q[:, :, 1] = ext[:, :TW, 1:TW + 1].transpose(1, 2, 0)
    q[:, :, 2] = ext[:, 1:TW + 1, :TW].transpose(1, 2, 0)
    q[:, :, 3] = ext[:, 1:TW + 1, 1:TW + 1].transpose(1, 2, 0)
    xq = q.reshape(TROWS, QELEM)

    # offset+mask conv weights -> [128, tap, cblk, 27] bf16 (partition-major)
    wom = np.concatenate([off_w, mask_w], axis=0)          # [27, 256, 3, 3]
    womT = (wom.reshape(27, 2, 128, K * K).transpose(2, 3, 1, 0)
            .astype(ml_dtypes.bfloat16).copy())
    ob = np.concatenate([off_b, mask_b])[:, None].astype(np.float32)

    # main weight -> [128, tap, cblk, 256o] bf16 (partition-major)
    wcT = (weight.reshape(256, 2, 128, K * K).transpose(2, 3, 1, 0)
           .astype(ml_dtypes.bfloat16).copy())

    # BY/BX [128, 2, NPB, 9]
    p = np.arange(HW)
    oy, ox = p // W, p % W
    ky, kx = np.meshgrid(np.arange(K), np.arange(K), indexing="ij")
    BY = (oy[:, None] + ky.reshape(-1)[None, :] - 1).astype(np.float32)
    BX = (ox[:, None] + kx.reshape(-1)[None, :] - 1).astype(np.float32)
    byx = (np.stack([BY, BX]).reshape(2, NPB, 128, NTAP)
           .transpose(2, 0, 1, 3).copy())

    idb = np.eye(128, dtype=np.float32).astype(ml_dtypes.bfloat16)

    return dict(xpad=xpad_bf, xq=xq, womT=womT, ob=ob, wcT=wcT, byx=byx,
                idb=idb)


_CACHED = {}


def kernel(x, weight, off_w, off_b, mask_w, mask_b, _stage="full", _trace=False):
    x = np.asarray(x, np.float32)
    weight = np.asarray(weight, np.float32)
    off_w = np.asarray(off_w, np.float32)
    off_b = np.asarray(off_b, np.float32)
    mask_w = np.asarray(mask_w, np.float32)
    mask_b = np.asarray(mask_b, np.float32)
    B = x.shape[0]
    assert B == 8

    if _stage not in _CACHED:
        _CACHED[_stage] = build_kernel(_stage)
    nc = _CACHED[_stage]

    in_maps = [
        _prep_core_inputs(x[b], weight, off_w, off_b, mask_w, mask_b)
        for b in range(B)
    ]
    res = run_bass_kernel_spmd(nc, in_maps, core_ids=list(range(8)), trace=_trace)
    if _stage != "full":
        return res

    out = np.empty((B, COUT, H, W), np.float32)
    for b in range(B):
        o = res.results[b]["out"]           # [2, 128, HW]
        out[b] = o.reshape(COUT, H, W)
    kernel._last_exec_time_ns = res.exec_time_ns
    return out
